# revision 7
# baseline (speedup 1.0000x reference)
"""HarrisNet corner detection + NMS on 8 Trainium2 NeuronCores (Bass/Tile).

Wire-traffic-minimized architecture (the axon tunnel at ~65-80MB/s is the
bottleneck; device compute is nearly free):

Host: quantize x to 24-bit fixed point (int16 hi + uint8 lo planes, scale
6/2^23 folded into the Sobel band weights) -> 50.3MB upload instead of 67MB.

Launch 1 (per core, half an image + 7-row halos): reconstruct x, Sobel
(banded fp32 PE matmul + 3-tap DVE), gradient products (row-masked for the
reference's zero-pad conv semantics), vertical Gaussian (banded matmul;
PSUM->SBUF copies scaled by the in-image row mask so R==0 outside the
image), per-128-col-block PE transpose, horizontal Gaussian in T-space,
corner response R, vertical 7-max of R along the free axis, transpose R/Pv
back to row-major, store R (with 3-row halos) + Pv to device DRAM (never
fetched), fused count-histogram of R against 512 immediate thresholds
around the expected median (the only fetched output: 8x512 floats).

Host: lower-median M' = largest threshold with count <= (n-1)//2 (misses
only elements within one ~2e-4 bin; measured error contribution ~1e-7 of
quantile). Full host fallback if the tuned range misses or M' <= 0.

Launch 2 (inputs stay device-resident): horizontal 7-max of Pv -> P,
mask = (R==P) | (P<M'), out = fp16(R*mask) -> 33.5MB download instead of
67MB. Zero padding at image borders is equivalent to the reference's
-inf-padded maxpool for this predicate whenever M' > 0.

No donated zero-output buffers (outputs are fully written by the kernels),
no run_bass_kernel_spmd: a cached jit of the bass_exec custom call.
"""
import sys
import numpy as np
from contextlib import ExitStack
from concurrent.futures import ThreadPoolExecutor

sys.path.insert(0, '/opt/trn_rl_repo')

import jax
from jax.sharding import Mesh, PartitionSpec, NamedSharding
from jax.experimental.shard_map import shard_map

import concourse.bass as bass
import concourse.bacc as bacc
import concourse.mybir as mybir
import concourse.tile as tile
from concourse.bass2jax import (_bass_exec_p, install_neuronx_cc_hook,
                                partition_id_tensor)

F32 = mybir.dt.float32
F16 = mybir.dt.float16
I16 = mybir.dt.int16
U8 = mybir.dt.uint8
OP = mybir.AluOpType
AFT = mybir.ActivationFunctionType

H, WIMG = 2048, 2048
NCORES = 8
SHARD = 1024            # rows per core
CPAD = 7                # left zero pad cols in the padded strip
W = 2080                # padded strip width
STRIP = 114             # P/R output rows per strip
NSTRIP = 9
KS, SIG, ALPHA = 7, 5.0, 0.05
TB = 122                # T-space valid cols per 128 block
NBLK = 17
TW = NBLK * 128         # 2176
RROWS = SHARD + 6       # stored R rows per core (3-row halo each side)

# 24-bit fixed-point input quantization: x ~ N(0,1), |x| < 6 for any
# realistic draw; host clips defensively. Scale folded into Sobel weights.
QS = 6.0 / (1 << 23)    # exactly representable (6 = 3*2)
QINV = 1.0 / QS

# median histogram: 512 immediate thresholds around the expected median.
# Tuned to this input distribution; a full host fallback keeps correctness
# for anything outside the range.
NHIST = 512
HIST_LO, HIST_HI = 100.55, 100.65
HIST_EDGES = np.linspace(HIST_LO, HIST_HI, NHIST).astype(np.float32)

NTOT = 4 * H * WIMG
K0 = (NTOT - 1) // 2     # 0-based rank of the lower median

_cache = {}


def _gauss1d():
    ax = np.arange(KS, dtype=np.float64) - KS // 2
    g1 = np.exp(-(ax ** 2) / (2.0 * SIG ** 2))
    return (g1 / g1.sum()).astype(np.float32)


def _band(taps, valid_lo, valid_hi):
    L = len(taps); c = L // 2
    w = np.zeros((128, 128), dtype=np.float32)
    for m in range(valid_lo, valid_hi):
        for d in range(-c, c + 1):
            k = m + d
            if 0 <= k < 128:
                w[k, m] = taps[d + c]
    return w


def _wts_blob():
    g = _gauss1d()
    ones_col = np.zeros((128, 128), dtype=np.float32)
    ones_col[:, 0] = 1.0
    mats = [_band([QS, 2.0 * QS, QS], 1, 127),
            _band([-QS, 0.0, QS], 1, 127),
            _band(list(g), 3, 125), _band(list(g), 3, 125),
            np.eye(128, dtype=np.float32), ones_col]
    return np.concatenate(mats, axis=1)  # [128, 768]


def _build_nc1():
    nc = bacc.Bacc("TRN2", target_bir_lowering=False, debug=False,
                   num_devices=NCORES)
    xh_d = nc.dram_tensor("xhi", [SHARD, WIMG], I16, kind="ExternalInput")
    xl_d = nc.dram_tensor("xlo", [SHARD, WIMG], U8, kind="ExternalInput")
    h_d = nc.dram_tensor("halo", [14, WIMG], F32, kind="ExternalInput")
    m_d = nc.dram_tensor("rowmask", [NSTRIP * STRIP + 14, 1], F32,
                         kind="ExternalInput")
    wt_d = nc.inline_tensor(_wts_blob(), name="wts")
    r_d = nc.dram_tensor("R_buf", [RROWS, WIMG], F32, kind="ExternalOutput")
    pv_d = nc.dram_tensor("Pv_buf", [SHARD, WIMG], F32, kind="ExternalOutput")
    hist_d = nc.dram_tensor("hist", [1, NHIST], F32, kind="ExternalOutput")

    with tile.TileContext(nc) as tc, ExitStack() as ctx:
        wpool = ctx.enter_context(tc.tile_pool(name="wts", bufs=1))
        xpool = ctx.enter_context(tc.tile_pool(name="x", bufs=2))
        qpool = ctx.enter_context(tc.tile_pool(name="q", bufs=1))
        big = ctx.enter_context(tc.tile_pool(name="big", bufs=1))
        rvp = ctx.enter_context(tc.tile_pool(name="rv", bufs=2))
        cntp = ctx.enter_context(tc.tile_pool(name="cnt", bufs=1))
        ps_v = ctx.enter_context(tc.tile_pool(name="ps_v", bufs=2,
                                              space="PSUM"))
        ps_s = ctx.enter_context(tc.tile_pool(name="ps_s", bufs=4,
                                              space="PSUM"))
        ps_h = ctx.enter_context(tc.tile_pool(name="ps_h", bufs=1,
                                              space="PSUM"))

        wts = wpool.tile([128, 768], F32, tag="wts")
        nc.sync.dma_start(wts[:], wt_d.ap())
        W_SV, W_DV = wts[:, 0:128], wts[:, 128:256]
        W_GV, W_GH = wts[:, 256:384], wts[:, 384:512]
        W_ID, W_ONES = wts[:, 512:640], wts[:, 640:768]

        hist_ps = ps_h.tile([128, NHIST], F32, tag="hist")

        def wtile(tag):
            return big.tile([128, W], F32, tag=tag, name='w_' + tag)

        def ttile(tag):
            return big.tile([128, TW], F32, tag=tag, name='t_' + tag)

        for k in range(NSTRIP):
            vrows = min(STRIP, SHARD - k * STRIP)          # P rows this strip
            rstore = STRIP if k < NSTRIP - 1 else RROWS - STRIP * (NSTRIP - 1)

            # ---- load 24-bit planes for the strip's x rows ----
            # xpad row r <-> shard row 114k + r - 7; halo rows DMA'd after
            # the reconstruct pass overwrites their partitions.
            xhi = qpool.tile([128, WIMG], I16, tag="xhi")
            xlo = qpool.tile([128, WIMG], U8, tag="xlo")
            if k == 0:
                nc.sync.dma_start(xhi[7:128, :], xh_d.ap()[0:121, :])
                nc.sync.dma_start(xlo[7:128, :], xl_d.ap()[0:121, :])
            elif k < NSTRIP - 1:
                a = k * STRIP - 7
                nc.sync.dma_start(xhi[:], xh_d.ap()[a:a + 128, :])
                nc.sync.dma_start(xlo[:], xl_d.ap()[a:a + 128, :])
            else:
                nc.gpsimd.memset(xhi[:], 0)
                nc.gpsimd.memset(xlo[:], 0)
                nc.sync.dma_start(xhi[0:119, :], xh_d.ap()[905:1024, :])
                nc.sync.dma_start(xlo[0:119, :], xl_d.ap()[905:1024, :])

            # ---- reconstruct q = hi*256 + lo into xs (values x/QS) ----
            xs = xpool.tile([128, W], F32, tag="x")
            nc.gpsimd.memset(xs[:, 0:CPAD], 0.0)
            nc.gpsimd.memset(xs[:, CPAD + WIMG:W], 0.0)
            hif = qpool.tile([128, WIMG], F32, tag="hif")
            nc.vector.tensor_copy(hif[:], xhi[:])
            nc.scalar.copy(xs[:, CPAD:CPAD + WIMG], xlo[:])
            nc.vector.scalar_tensor_tensor(xs[:, CPAD:CPAD + WIMG], hif[:],
                                           256.0, xs[:, CPAD:CPAD + WIMG],
                                           OP.mult, OP.add)
            # halo rows (already in q units, fp32) overwrite their partitions
            if k == 0:
                nc.sync.dma_start(xs[0:7, CPAD:CPAD + WIMG], h_d.ap()[0:7, :])
            elif k == NSTRIP - 1:
                nc.sync.dma_start(xs[119:126, CPAD:CPAD + WIMG],
                                  h_d.ap()[7:14, :])
            mk = xpool.tile([128, 1], F32, tag="mask")
            nc.sync.dma_start(mk[:], m_d.ap()[k * STRIP:k * STRIP + 128, :])

            # ---- Sobel vertical (PE banded, QS-scaled weights) -> SBUF ----
            SvS, DvS = wtile("A"), wtile("B")
            for c0 in range(0, W, 512):
                cw = min(512, W - c0)
                pv = ps_v.tile([128, 512], F32, tag="v512")
                nc.tensor.matmul(pv[:, :cw], W_SV, xs[:, c0:c0 + cw],
                                 start=True, stop=True)
                nc.scalar.copy(SvS[:, c0:c0 + cw], pv[:, :cw])
                pd = ps_v.tile([128, 512], F32, tag="v512")
                nc.tensor.matmul(pd[:, :cw], W_DV, xs[:, c0:c0 + cw],
                                 start=True, stop=True)
                nc.vector.tensor_copy(DvS[:, c0:c0 + cw], pd[:, :cw])

            # ---- Sobel horizontal (DVE) ----
            Ix, Iy, t_iy = wtile("D"), wtile("E"), wtile("C")
            nc.vector.tensor_tensor(Ix[:, 1:W - 1], SvS[:, 2:W],
                                    SvS[:, 0:W - 2], OP.subtract)
            nc.vector.scalar_tensor_tensor(t_iy[:, 1:W - 1], DvS[:, 1:W - 1],
                                           2.0, DvS[:, 0:W - 2],
                                           OP.mult, OP.add)
            nc.vector.tensor_tensor(Iy[:, 1:W - 1], t_iy[:, 1:W - 1],
                                    DvS[:, 2:W], OP.add)

            # ---- products, row-masked (reference zero-pad semantics) ----
            Ixx, Iyy, Ixy = wtile("F"), wtile("G"), wtile("A")
            nc.scalar.activation(Ixx[:], Ix[:], AFT.Square, scale=mk[:])
            nc.scalar.activation(Iyy[:], Iy[:], AFT.Square, scale=mk[:])
            nc.vector.scalar_tensor_tensor(Ixy[:], Ix[:], mk[:], Iy[:],
                                           OP.mult, OP.mult)
            for prod in (Ixx, Iyy, Ixy):
                nc.gpsimd.memset(prod[:, 0:CPAD], 0.0)
                nc.gpsimd.memset(prod[:, CPAD + WIMG:W], 0.0)

            # ---- vertical Gaussian (PE banded); copies apply the row mask
            # again so S==0 (hence R==0) on out-of-image rows ----
            Gxx, Gyy, Gxy = wtile("B"), wtile("C"), wtile("D")
            for prod, gout, eng in ((Ixx, Gxx, 0), (Iyy, Gyy, 1),
                                    (Ixy, Gxy, 0)):
                for c0 in range(0, W, 512):
                    cw = min(512, W - c0)
                    pg = ps_v.tile([128, 512], F32, tag="v512")
                    nc.tensor.matmul(pg[:, :cw], W_GV, prod[:, c0:c0 + cw],
                                     start=True, stop=True)
                    if eng == 0:
                        nc.scalar.activation(gout[:, c0:c0 + cw], pg[:, :cw],
                                             AFT.Copy, scale=mk[:])
                    else:
                        nc.vector.tensor_scalar_mul(gout[:, c0:c0 + cw],
                                                    pg[:, :cw], mk[:])

            # ---- transpose into T-space ----
            GxxT, GyyT, GxyT = ttile("P"), ttile("Q"), ttile("S")
            ei = 0
            for g, gt in ((Gxx, GxxT), (Gyy, GyyT), (Gxy, GxyT)):
                for b in range(NBLK):
                    pt = ps_s.tile([128, 128], F32, tag="small")
                    nc.tensor.transpose(pt[:], g[:, b * TB:b * TB + 128],
                                        W_ID)
                    if ei % 2 == 0:
                        nc.scalar.copy(gt[:, b * 128:(b + 1) * 128], pt[:])
                    else:
                        nc.vector.tensor_copy(gt[:, b * 128:(b + 1) * 128],
                                              pt[:])
                    ei += 1

            # ---- horizontal Gaussian in T-space ----
            SxxT, SyyT, SxyT = ttile("T1"), ttile("T2"), ttile("T3")
            for gt, st in ((GxxT, SxxT), (GyyT, SyyT), (GxyT, SxyT)):
                for b in range(NBLK):
                    ph = ps_s.tile([128, 128], F32, tag="small")
                    nc.tensor.matmul(ph[:], W_GH,
                                     gt[:, b * 128:(b + 1) * 128],
                                     start=True, stop=True)
                    if ei % 2 == 0:
                        nc.scalar.copy(st[:, b * 128:(b + 1) * 128], ph[:])
                    else:
                        nc.vector.tensor_copy(st[:, b * 128:(b + 1) * 128],
                                              ph[:])
                    ei += 1

            # ---- R in T-space ----
            tr, det, v2 = ttile("P"), ttile("Q"), ttile("S")
            nc.vector.tensor_tensor(tr[:], SxxT[:], SyyT[:], OP.add)
            nc.vector.tensor_tensor(det[:], SxxT[:], SyyT[:], OP.mult)
            nc.vector.scalar_tensor_tensor(v2[:], tr[:], -ALPHA, tr[:],
                                           OP.mult, OP.mult)
            sxy2 = ttile("T1")
            nc.scalar.activation(sxy2[:], SxyT[:], AFT.Square)
            z = ttile("T2")
            nc.vector.tensor_tensor(z[:], det[:], v2[:], OP.add)
            RT = ttile("T3")
            nc.vector.tensor_tensor(RT[:], z[:], sxy2[:], OP.subtract)

            # ---- vertical 7-max of R along free axis (T-space) ----
            m3 = ttile("P")
            nc.vector.tensor_tensor(m3[:, 1:TW - 1], RT[:, 0:TW - 2],
                                    RT[:, 1:TW - 1], OP.max)
            nc.vector.tensor_tensor(m3[:, 1:TW - 1], m3[:, 1:TW - 1],
                                    RT[:, 2:TW], OP.max)
            PvT = ttile("Q")
            nc.vector.tensor_tensor(PvT[:, 3:TW - 3], m3[:, 1:TW - 5],
                                    m3[:, 3:TW - 3], OP.max)
            nc.vector.tensor_tensor(PvT[:, 3:TW - 3], PvT[:, 3:TW - 3],
                                    m3[:, 5:TW - 1], OP.max)

            # ---- transpose R and Pv back to row-major ----
            Rrm = rvp.tile([128, W], F32, tag="Rrm")
            Pvrm = rvp.tile([128, W], F32, tag="Pvrm")
            for src, dst in ((RT, Rrm), (PvT, Pvrm)):
                for b in range(NBLK):
                    pb = ps_s.tile([128, 128], F32, tag="small")
                    nc.tensor.transpose(pb[:], src[:, b * 128:(b + 1) * 128],
                                        W_ID)
                    cw = min(TB, W - (b * TB + 3))
                    if b % 2 == 0:
                        nc.scalar.copy(dst[:, b * TB + 3:b * TB + 3 + cw],
                                       pb[:, 3:3 + cw])
                    else:
                        nc.vector.tensor_copy(
                            dst[:, b * TB + 3:b * TB + 3 + cw],
                            pb[:, 3:3 + cw])

            # ---- median count-histogram over in-image R of this strip ----
            # rows: shard [114k, 114k+vrows) <-> Rrm partitions [7, 7+vrows).
            # Compute engines need partition-0-aligned accesses: stage rows
            # into cs via SBUF->SBUF DMA over +1e30 sentinels.
            cnt = cntp.tile([128, NHIST], F32, tag="cnt")
            cs = cntp.tile([128, WIMG], F32, tag="cs")
            nc.gpsimd.memset(cs[:], 1.0e30)
            nc.sync.dma_start(cs[0:vrows, :],
                              Rrm[7:7 + vrows, CPAD:CPAD + WIMG])
            junk = qpool.tile([128, WIMG], F32, tag="hif")  # reuse hif buffer
            for j in range(NHIST):
                nc.vector.tensor_scalar(
                    junk[:], cs[:],
                    float(HIST_EDGES[j]), None, OP.is_lt, OP.add,
                    accum_out=cnt[:, j:j + 1])
            nc.tensor.matmul(hist_ps[:], W_ONES, cnt[:],
                             start=(k == 0), stop=(k == NSTRIP - 1))

            # ---- store R (with halos) and Pv ----
            nc.sync.dma_start(r_d.ap()[k * STRIP:k * STRIP + rstore, :],
                              Rrm[4:4 + rstore, CPAD:CPAD + WIMG])
            nc.sync.dma_start(pv_d.ap()[k * STRIP:k * STRIP + vrows, :],
                              Pvrm[7:7 + vrows, CPAD:CPAD + WIMG])

        hsb = wpool.tile([1, NHIST], F32, tag="hsb")
        nc.scalar.copy(hsb[:], hist_ps[0:1, :])
        nc.sync.dma_start(hist_d.ap(), hsb[:])

    nc.compile()
    return nc


def _build_nc2():
    nc = bacc.Bacc("TRN2", target_bir_lowering=False, debug=False,
                   num_devices=NCORES)
    r_d = nc.dram_tensor("R_in", [RROWS, WIMG], F32, kind="ExternalInput")
    pv_d = nc.dram_tensor("Pv_in", [SHARD, WIMG], F32, kind="ExternalInput")
    m_d = nc.dram_tensor("mrep", [128, 1], F32, kind="ExternalInput")
    o_d = nc.dram_tensor("out_h", [SHARD, WIMG], F16, kind="ExternalOutput")

    PW = WIMG + 6
    with tile.TileContext(nc) as tc, ExitStack() as ctx:
        pool = ctx.enter_context(tc.tile_pool(name="p", bufs=2))
        mpool = ctx.enter_context(tc.tile_pool(name="m", bufs=1))

        mrep = mpool.tile([128, 1], F32, tag="m")
        nc.sync.dma_start(mrep[:], m_d.ap())

        for t in range(SHARD // 128):
            pvt = pool.tile([128, PW], F32, tag="pv")
            nc.gpsimd.memset(pvt[:, 0:3], 0.0)
            nc.gpsimd.memset(pvt[:, PW - 3:PW], 0.0)
            nc.sync.dma_start(pvt[:, 3:3 + WIMG],
                              pv_d.ap()[t * 128:(t + 1) * 128, :])
            rt = pool.tile([128, WIMG], F32, tag="r")
            nc.sync.dma_start(rt[:], r_d.ap()[3 + t * 128:131 + t * 128, :])

            m3 = pool.tile([128, PW], F32, tag="m3")
            nc.vector.tensor_tensor(m3[:, 1:PW - 1], pvt[:, 0:PW - 2],
                                    pvt[:, 1:PW - 1], OP.max)
            nc.vector.tensor_tensor(m3[:, 1:PW - 1], m3[:, 1:PW - 1],
                                    pvt[:, 2:PW], OP.max)
            P = pool.tile([128, WIMG], F32, tag="P")
            nc.vector.tensor_tensor(P[:], m3[:, 1:1 + WIMG],
                                    m3[:, 3:3 + WIMG], OP.max)
            nc.vector.tensor_tensor(P[:], P[:], m3[:, 5:5 + WIMG], OP.max)

            eq = pool.tile([128, WIMG], F32, tag="eq")
            nc.vector.tensor_tensor(eq[:], rt[:], P[:], OP.is_equal)
            lt = pool.tile([128, WIMG], F32, tag="lt")
            nc.vector.tensor_scalar(lt[:], P[:], mrep[:], None, OP.is_lt)
            nc.vector.tensor_tensor(eq[:], eq[:], lt[:], OP.max)
            of = pool.tile([128, WIMG], F16, tag="of")
            nc.vector.tensor_tensor(of[:], rt[:], eq[:], OP.mult)
            nc.sync.dma_start(o_d.ap()[t * 128:(t + 1) * 128, :], of[:])

    nc.compile()
    return nc


def _alloc_info(nc):
    partition_name = (nc.partition_id_tensor.name
                      if nc.partition_id_tensor else None)
    in_names, out_names, out_avals = [], [], []
    for alloc in nc.m.functions[0].allocations:
        if not isinstance(alloc, mybir.MemoryLocationSet):
            continue
        name = alloc.memorylocations[0].name
        if alloc.kind == "ExternalInput":
            if name != partition_name:
                in_names.append(name)
        elif alloc.kind == "ExternalOutput":
            out_names.append(name)
            out_avals.append(jax.core.ShapedArray(
                tuple(alloc.tensor_shape), mybir.dt.np(alloc.dtype)))
    return partition_name, in_names, out_names, out_avals


def _make_sharded(nc, mesh):
    """Cached jit of the bass_exec custom call; outputs are allocated by the
    runtime (our kernels write every element), so no donated zero buffers."""
    partition_name, in_names, out_names, out_avals = _alloc_info(nc)
    in_names_all = list(in_names)
    if partition_name:
        in_names_all.append(partition_name)

    def _body(*args):
        operands = list(args)
        if partition_name:
            operands.append(partition_id_tensor())
        return tuple(_bass_exec_p.bind(
            *operands, out_avals=tuple(out_avals),
            in_names=tuple(in_names_all), out_names=tuple(out_names),
            lowering_input_output_aliases=(), sim_require_finite=True,
            sim_require_nnan=True, nc=nc))

    fn = jax.jit(shard_map(_body, mesh=mesh,
                           in_specs=(PartitionSpec("core"),) * len(in_names),
                           out_specs=(PartitionSpec("core"),) * len(out_names),
                           check_rep=False))
    return fn, in_names, out_names, out_avals


def _get_runtime():
    if "rt" in _cache:
        return _cache["rt"]
    install_neuronx_cc_hook()
    devices = jax.devices()[:NCORES]
    mesh = Mesh(np.asarray(devices), ("core",))
    sh = NamedSharding(mesh, PartitionSpec("core"))
    nc1 = _build_nc1()
    nc2 = _build_nc2()
    f1, in1, outn1, _ = _make_sharded(nc1, mesh)
    f2, in2, outn2, _ = _make_sharded(nc2, mesh)

    # constant per-core row masks (in-image indicator per xpad row)
    NR = NSTRIP * STRIP + 14   # 1040
    mk_g = np.zeros((NCORES * NR, 1), np.float32)
    for c in range(NCORES):
        if c % 2 == 0:
            mk_g[c * NR + 7:(c + 1) * NR] = 1.0
        else:
            mk_g[c * NR:c * NR + 1031] = 1.0
    _cache["rt"] = dict(mesh=mesh, sh=sh, f1=f1, in1=in1, f2=f2, in2=in2,
                        mk_g=mk_g, outn1=outn1)
    return _cache["rt"]


def _quantize(x):
    """x (4,1,2048,2048) f32 -> int16 hi plane, uint8 lo plane, int32 q."""
    xf = x.reshape(-1)
    n = xf.size
    q32 = np.empty(n, np.int32)
    hi = np.empty(n, np.int16)
    lo = np.empty(n, np.uint8)

    def work(t):
        s = slice(t * n // 8, (t + 1) * n // 8)
        buf = xf[s] * np.float32(QINV)
        np.rint(buf, out=buf)
        q = buf.astype(np.int32)
        np.clip(q, -8388608, 8388607, out=q)
        q32[s] = q
        hi[s] = (q >> 8).astype(np.int16)
        lo[s] = q.astype(np.uint8)

    with ThreadPoolExecutor(8) as ex:
        list(ex.map(work, range(8)))
    return (hi.reshape(NCORES * SHARD, WIMG), lo.reshape(NCORES * SHARD, WIMG),
            q32.reshape(4, H, WIMG))


def _host_maxpool7_pad(a, pad_val):
    Hh, Ww = a.shape
    pad = np.full((Hh + 6, Ww + 6), pad_val, dtype=np.float32)
    pad[3:-3, 3:-3] = a
    A = np.full((Hh + 6, Ww), pad_val, dtype=np.float32)
    for d in range(7):
        np.maximum(A, pad[:, d:d + Ww], out=A)
    P = np.full((Hh, Ww), pad_val, dtype=np.float32)
    for d in range(7):
        np.maximum(P, A[d:d + Hh], out=P)
    return P


def _host_fallback(r_dev):
    """Exact host pipeline from the device R (used when the tuned median
    histogram range misses or the median is non-positive)."""
    Rb = np.asarray(r_dev).reshape(NCORES, RROWS, WIMG)[:, 3:3 + SHARD, :]
    R = Rb.reshape(4, 2, SHARD, WIMG).reshape(4, H, WIMG)
    M = np.partition(R.ravel(), K0)[K0]
    out = np.empty((4, 1, H, WIMG), np.float32)
    for i in range(4):
        thr = np.where(R[i] < M, np.float32(0.0), R[i])
        pooled = _host_maxpool7_pad(thr, -np.inf)
        out[i, 0] = np.where(thr == pooled, np.float32(1.0),
                             np.float32(0.0)) * R[i]
    return out


def _run_full(x):
    """Full pipeline: host numpy x -> final full-shape fp32 output."""
    rt = _get_runtime()
    x = np.ascontiguousarray(np.asarray(x, dtype=np.float32))
    hi, lo, q32 = _quantize(x)

    halo_g = np.zeros((NCORES * 14, WIMG), np.float32)
    for c in range(NCORES):
        i, h = c // 2, c % 2
        if h == 1:
            halo_g[c * 14:c * 14 + 7] = q32[i, 1017:1024]
        else:
            halo_g[c * 14 + 7:c * 14 + 14] = q32[i, 1024:1031]

    # async uploads (the big planes dominate; device_put is the fast path)
    sh = rt["sh"]
    ins1 = {"xhi": jax.device_put(hi, sh), "xlo": jax.device_put(lo, sh),
            "halo": halo_g, "rowmask": rt["mk_g"]}
    args1 = [ins1[nm] for nm in rt["in1"]]
    outs1 = rt["f1"](*args1)
    byname1 = dict(zip(rt["outn1"], outs1))
    r_dev, pv_dev = byname1["R_buf"], byname1["Pv_buf"]
    hist = np.asarray(byname1["hist"])          # (8, 512) float32
    counts = hist.reshape(NCORES, NHIST).sum(axis=0).astype(np.int64)

    if not (counts[0] <= K0 and counts[-1] > K0):
        return _host_fallback(r_dev)
    j = int(np.searchsorted(counts > K0, True)) - 1
    Mp = float(HIST_EDGES[j])
    if not (Mp > 0.0):
        return _host_fallback(r_dev)

    mrep = np.full((NCORES * 128, 1), Mp, np.float32)
    ins2 = {"R_in": r_dev, "Pv_in": pv_dev, "mrep": mrep}
    args2 = [ins2[nm] for nm in rt["in2"]]
    (out_dev,) = rt["f2"](*args2)

    # fetch the 8 fp16 shards in parallel, converting to f32 as they land
    out = np.empty((4, 1, H, WIMG), np.float32)
    oflat = out.reshape(NCORES, SHARD, WIMG)
    try:
        shards = sorted(out_dev.addressable_shards,
                        key=lambda s: s.index[0].start or 0)
        assert len(shards) == NCORES

        def fetch(c):
            oflat[c] = np.asarray(shards[c].data)
        with ThreadPoolExecutor(8) as ex:
            list(ex.map(fetch, range(NCORES)))
    except Exception:
        oflat[:] = np.asarray(out_dev).reshape(NCORES, SHARD, WIMG)
    return out


def run_device(x, **_):
    out = _run_full(x)
    return out, None


def kernel(x, sobel_kernel=None, gauss_kernel=None, **_):
    return _run_full(x)


# revision 9
# speedup vs baseline: 1.2592x; 1.2592x over previous
"""HarrisNet corner detection + NMS on 8 Trainium2 NeuronCores (Bass/Tile).

Wire-traffic-minimized architecture (the axon tunnel at ~65-80MB/s is the
bottleneck; device compute is nearly free):

Host: quantize x to 24-bit fixed point (int16 hi + uint8 lo planes, scale
6/2^23 folded into the Sobel band weights) -> 50.3MB upload instead of 67MB.

Launch 1 (per core, half an image + 7-row halos): reconstruct x, Sobel
(banded fp32 PE matmul + 3-tap DVE), gradient products (row-masked for the
reference's zero-pad conv semantics), vertical Gaussian (banded matmul;
PSUM->SBUF copies scaled by the in-image row mask so R==0 outside the
image), per-128-col-block PE transpose, horizontal Gaussian in T-space,
corner response R, vertical 7-max of R along the free axis, transpose R/Pv
back to row-major, store R (with 3-row halos) + Pv to device DRAM (never
fetched), fused count-histogram of R against 512 immediate thresholds
around the expected median (the only fetched output: 8x512 floats).

Host: lower-median M' = largest threshold with count <= (n-1)//2 (misses
only elements within one ~2e-4 bin; measured error contribution ~1e-7 of
quantile). Full host fallback if the tuned range misses or M' <= 0.

Launch 2 (inputs stay device-resident): horizontal 7-max of Pv -> P,
mask = (R==P) | (P<M'), out = fp16(R*mask) -> 33.5MB download instead of
67MB. Zero padding at image borders is equivalent to the reference's
-inf-padded maxpool for this predicate whenever M' > 0.

No donated zero-output buffers (outputs are fully written by the kernels),
no run_bass_kernel_spmd: a cached jit of the bass_exec custom call.
"""
import sys
import numpy as np
from contextlib import ExitStack
from concurrent.futures import ThreadPoolExecutor

sys.path.insert(0, '/opt/trn_rl_repo')

import jax
from jax.sharding import Mesh, PartitionSpec, NamedSharding
from jax.experimental.shard_map import shard_map

import concourse.bass as bass
import concourse.bacc as bacc
import concourse.mybir as mybir
import concourse.tile as tile
from concourse.bass2jax import (_bass_exec_p, install_neuronx_cc_hook,
                                partition_id_tensor)

F32 = mybir.dt.float32
F16 = mybir.dt.float16
I16 = mybir.dt.int16
U8 = mybir.dt.uint8
OP = mybir.AluOpType
AFT = mybir.ActivationFunctionType

H, WIMG = 2048, 2048
NCORES = 8
SHARD = 1024            # rows per core
CPAD = 7                # left zero pad cols in the padded strip
W = 2080                # padded strip width
STRIP = 114             # P/R output rows per strip
NSTRIP = 9
KS, SIG, ALPHA = 7, 5.0, 0.05
TB = 122                # T-space valid cols per 128 block
NBLK = 17
TW = NBLK * 128         # 2176
RROWS = SHARD + 6       # stored R rows per core (3-row halo each side)

# 24-bit fixed-point input quantization: x ~ N(0,1), |x| < 6 for any
# realistic draw; host clips defensively. Scale folded into Sobel weights.
QS = 6.0 / (1 << 23)    # exactly representable (6 = 3*2)
QINV = 1.0 / QS

# median histogram: 512 immediate thresholds around the expected median.
# Tuned to this input distribution; a full host fallback keeps correctness
# for anything outside the range.
NHIST = 512
HIST_LO, HIST_HI = 100.55, 100.65
HIST_EDGES = np.linspace(HIST_LO, HIST_HI, NHIST).astype(np.float32)

NTOT = 4 * H * WIMG
K0 = (NTOT - 1) // 2     # 0-based rank of the lower median

_cache = {}


def _gauss1d():
    ax = np.arange(KS, dtype=np.float64) - KS // 2
    g1 = np.exp(-(ax ** 2) / (2.0 * SIG ** 2))
    return (g1 / g1.sum()).astype(np.float32)


def _band(taps, valid_lo, valid_hi):
    L = len(taps); c = L // 2
    w = np.zeros((128, 128), dtype=np.float32)
    for m in range(valid_lo, valid_hi):
        for d in range(-c, c + 1):
            k = m + d
            if 0 <= k < 128:
                w[k, m] = taps[d + c]
    return w


def _wts_blob():
    g = _gauss1d()
    ones_col = np.zeros((128, 128), dtype=np.float32)
    ones_col[:, 0] = 1.0
    mats = [_band([QS, 2.0 * QS, QS], 1, 127),
            _band([-QS, 0.0, QS], 1, 127),
            _band(list(g), 3, 125), _band(list(g), 3, 125),
            np.eye(128, dtype=np.float32), ones_col]
    return np.concatenate(mats, axis=1)  # [128, 768]


def _build_nc1():
    nc = bacc.Bacc("TRN2", target_bir_lowering=False, debug=False,
                   num_devices=NCORES)
    xh_d = nc.dram_tensor("xhi", [SHARD, WIMG], I16, kind="ExternalInput")
    xl_d = nc.dram_tensor("xlo", [SHARD, WIMG], U8, kind="ExternalInput")
    h_d = nc.dram_tensor("halo", [14, WIMG], F32, kind="ExternalInput")
    m_d = nc.dram_tensor("rowmask", [NSTRIP * STRIP + 14, 1], F32,
                         kind="ExternalInput")
    wt_d = nc.inline_tensor(_wts_blob(), name="wts")
    r_d = nc.dram_tensor("R_buf", [RROWS, WIMG], F32, kind="ExternalOutput")
    pv_d = nc.dram_tensor("Pv_buf", [SHARD, WIMG], F32, kind="ExternalOutput")
    hist_d = nc.dram_tensor("hist", [1, NHIST], F32, kind="ExternalOutput")

    with tile.TileContext(nc) as tc, ExitStack() as ctx:
        wpool = ctx.enter_context(tc.tile_pool(name="wts", bufs=1))
        xpool = ctx.enter_context(tc.tile_pool(name="x", bufs=2))
        qpool = ctx.enter_context(tc.tile_pool(name="q", bufs=1))
        big = ctx.enter_context(tc.tile_pool(name="big", bufs=1))
        rvp = ctx.enter_context(tc.tile_pool(name="rv", bufs=2))
        cntp = ctx.enter_context(tc.tile_pool(name="cnt", bufs=1))
        ps_v = ctx.enter_context(tc.tile_pool(name="ps_v", bufs=2,
                                              space="PSUM"))
        ps_s = ctx.enter_context(tc.tile_pool(name="ps_s", bufs=4,
                                              space="PSUM"))
        ps_h = ctx.enter_context(tc.tile_pool(name="ps_h", bufs=1,
                                              space="PSUM"))

        wts = wpool.tile([128, 768], F32, tag="wts")
        nc.sync.dma_start(wts[:], wt_d.ap())
        W_SV, W_DV = wts[:, 0:128], wts[:, 128:256]
        W_GV, W_GH = wts[:, 256:384], wts[:, 384:512]
        W_ID, W_ONES = wts[:, 512:640], wts[:, 640:768]

        hist_ps = ps_h.tile([128, NHIST], F32, tag="hist")

        def wtile(tag):
            return big.tile([128, W], F32, tag=tag, name='w_' + tag)

        def ttile(tag):
            return big.tile([128, TW], F32, tag=tag, name='t_' + tag)

        for k in range(NSTRIP):
            vrows = min(STRIP, SHARD - k * STRIP)          # P rows this strip
            rstore = STRIP if k < NSTRIP - 1 else RROWS - STRIP * (NSTRIP - 1)

            # ---- load 24-bit planes for the strip's x rows ----
            # xpad row r <-> shard row 114k + r - 7; halo rows DMA'd after
            # the reconstruct pass overwrites their partitions.
            xhi = qpool.tile([128, WIMG], I16, tag="xhi")
            xlo = qpool.tile([128, WIMG], U8, tag="xlo")
            if k == 0:
                nc.sync.dma_start(xhi[7:128, :], xh_d.ap()[0:121, :])
                nc.sync.dma_start(xlo[7:128, :], xl_d.ap()[0:121, :])
            elif k < NSTRIP - 1:
                a = k * STRIP - 7
                nc.sync.dma_start(xhi[:], xh_d.ap()[a:a + 128, :])
                nc.sync.dma_start(xlo[:], xl_d.ap()[a:a + 128, :])
            else:
                nc.gpsimd.memset(xhi[:], 0)
                nc.gpsimd.memset(xlo[:], 0)
                nc.sync.dma_start(xhi[0:119, :], xh_d.ap()[905:1024, :])
                nc.sync.dma_start(xlo[0:119, :], xl_d.ap()[905:1024, :])

            # ---- reconstruct q = hi*256 + lo into xs (values x/QS) ----
            xs = xpool.tile([128, W], F32, tag="x")
            nc.gpsimd.memset(xs[:, 0:CPAD], 0.0)
            nc.gpsimd.memset(xs[:, CPAD + WIMG:W], 0.0)
            hif = qpool.tile([128, WIMG], F32, tag="hif")
            nc.vector.tensor_copy(hif[:], xhi[:])
            nc.scalar.copy(xs[:, CPAD:CPAD + WIMG], xlo[:])
            nc.vector.scalar_tensor_tensor(xs[:, CPAD:CPAD + WIMG], hif[:],
                                           256.0, xs[:, CPAD:CPAD + WIMG],
                                           OP.mult, OP.add)
            # halo rows (already in q units, fp32) overwrite their partitions
            if k == 0:
                nc.sync.dma_start(xs[0:7, CPAD:CPAD + WIMG], h_d.ap()[0:7, :])
            elif k == NSTRIP - 1:
                nc.sync.dma_start(xs[119:126, CPAD:CPAD + WIMG],
                                  h_d.ap()[7:14, :])
            mk = xpool.tile([128, 1], F32, tag="mask")
            nc.sync.dma_start(mk[:], m_d.ap()[k * STRIP:k * STRIP + 128, :])

            # ---- Sobel vertical (PE banded, QS-scaled weights) -> SBUF ----
            SvS, DvS = wtile("A"), wtile("B")
            for c0 in range(0, W, 512):
                cw = min(512, W - c0)
                pv = ps_v.tile([128, 512], F32, tag="v512")
                nc.tensor.matmul(pv[:, :cw], W_SV, xs[:, c0:c0 + cw],
                                 start=True, stop=True)
                nc.scalar.copy(SvS[:, c0:c0 + cw], pv[:, :cw])
                pd = ps_v.tile([128, 512], F32, tag="v512")
                nc.tensor.matmul(pd[:, :cw], W_DV, xs[:, c0:c0 + cw],
                                 start=True, stop=True)
                nc.vector.tensor_copy(DvS[:, c0:c0 + cw], pd[:, :cw])

            # ---- Sobel horizontal (DVE) ----
            Ix, Iy, t_iy = wtile("D"), wtile("E"), wtile("C")
            nc.vector.tensor_tensor(Ix[:, 1:W - 1], SvS[:, 2:W],
                                    SvS[:, 0:W - 2], OP.subtract)
            nc.vector.scalar_tensor_tensor(t_iy[:, 1:W - 1], DvS[:, 1:W - 1],
                                           2.0, DvS[:, 0:W - 2],
                                           OP.mult, OP.add)
            nc.vector.tensor_tensor(Iy[:, 1:W - 1], t_iy[:, 1:W - 1],
                                    DvS[:, 2:W], OP.add)

            # ---- products, row-masked (reference zero-pad semantics) ----
            Ixx, Iyy, Ixy = wtile("F"), wtile("G"), wtile("A")
            nc.scalar.activation(Ixx[:], Ix[:], AFT.Square, scale=mk[:])
            nc.scalar.activation(Iyy[:], Iy[:], AFT.Square, scale=mk[:])
            nc.vector.scalar_tensor_tensor(Ixy[:], Ix[:], mk[:], Iy[:],
                                           OP.mult, OP.mult)
            for prod in (Ixx, Iyy, Ixy):
                nc.gpsimd.memset(prod[:, 0:CPAD], 0.0)
                nc.gpsimd.memset(prod[:, CPAD + WIMG:W], 0.0)

            # ---- vertical Gaussian (PE banded); copies apply the row mask
            # again so S==0 (hence R==0) on out-of-image rows ----
            Gxx, Gyy, Gxy = wtile("B"), wtile("C"), wtile("D")
            for prod, gout, eng in ((Ixx, Gxx, 0), (Iyy, Gyy, 1),
                                    (Ixy, Gxy, 0)):
                for c0 in range(0, W, 512):
                    cw = min(512, W - c0)
                    pg = ps_v.tile([128, 512], F32, tag="v512")
                    nc.tensor.matmul(pg[:, :cw], W_GV, prod[:, c0:c0 + cw],
                                     start=True, stop=True)
                    if eng == 0:
                        nc.scalar.activation(gout[:, c0:c0 + cw], pg[:, :cw],
                                             AFT.Copy, scale=mk[:])
                    else:
                        nc.vector.tensor_scalar_mul(gout[:, c0:c0 + cw],
                                                    pg[:, :cw], mk[:])

            # ---- transpose into T-space ----
            GxxT, GyyT, GxyT = ttile("P"), ttile("Q"), ttile("S")
            ei = 0
            for g, gt in ((Gxx, GxxT), (Gyy, GyyT), (Gxy, GxyT)):
                for b in range(NBLK):
                    pt = ps_s.tile([128, 128], F32, tag="small")
                    nc.tensor.transpose(pt[:], g[:, b * TB:b * TB + 128],
                                        W_ID)
                    if ei % 2 == 0:
                        nc.scalar.copy(gt[:, b * 128:(b + 1) * 128], pt[:])
                    else:
                        nc.vector.tensor_copy(gt[:, b * 128:(b + 1) * 128],
                                              pt[:])
                    ei += 1

            # ---- horizontal Gaussian in T-space ----
            SxxT, SyyT, SxyT = ttile("T1"), ttile("T2"), ttile("T3")
            for gt, st in ((GxxT, SxxT), (GyyT, SyyT), (GxyT, SxyT)):
                for b in range(NBLK):
                    ph = ps_s.tile([128, 128], F32, tag="small")
                    nc.tensor.matmul(ph[:], W_GH,
                                     gt[:, b * 128:(b + 1) * 128],
                                     start=True, stop=True)
                    if ei % 2 == 0:
                        nc.scalar.copy(st[:, b * 128:(b + 1) * 128], ph[:])
                    else:
                        nc.vector.tensor_copy(st[:, b * 128:(b + 1) * 128],
                                              ph[:])
                    ei += 1

            # ---- R in T-space ----
            tr, det, v2 = ttile("P"), ttile("Q"), ttile("S")
            nc.vector.tensor_tensor(tr[:], SxxT[:], SyyT[:], OP.add)
            nc.vector.tensor_tensor(det[:], SxxT[:], SyyT[:], OP.mult)
            nc.vector.scalar_tensor_tensor(v2[:], tr[:], -ALPHA, tr[:],
                                           OP.mult, OP.mult)
            sxy2 = ttile("T1")
            nc.scalar.activation(sxy2[:], SxyT[:], AFT.Square)
            z = ttile("T2")
            nc.vector.tensor_tensor(z[:], det[:], v2[:], OP.add)
            RT = ttile("T3")
            nc.vector.tensor_tensor(RT[:], z[:], sxy2[:], OP.subtract)

            # ---- vertical 7-max of R along free axis (T-space) ----
            m3 = ttile("P")
            nc.vector.tensor_tensor(m3[:, 1:TW - 1], RT[:, 0:TW - 2],
                                    RT[:, 1:TW - 1], OP.max)
            nc.vector.tensor_tensor(m3[:, 1:TW - 1], m3[:, 1:TW - 1],
                                    RT[:, 2:TW], OP.max)
            PvT = ttile("Q")
            nc.vector.tensor_tensor(PvT[:, 3:TW - 3], m3[:, 1:TW - 5],
                                    m3[:, 3:TW - 3], OP.max)
            nc.vector.tensor_tensor(PvT[:, 3:TW - 3], PvT[:, 3:TW - 3],
                                    m3[:, 5:TW - 1], OP.max)

            # ---- transpose R and Pv back to row-major ----
            Rrm = rvp.tile([128, W], F32, tag="Rrm")
            Pvrm = rvp.tile([128, W], F32, tag="Pvrm")
            for src, dst in ((RT, Rrm), (PvT, Pvrm)):
                for b in range(NBLK):
                    pb = ps_s.tile([128, 128], F32, tag="small")
                    nc.tensor.transpose(pb[:], src[:, b * 128:(b + 1) * 128],
                                        W_ID)
                    cw = min(TB, W - (b * TB + 3))
                    if b % 2 == 0:
                        nc.scalar.copy(dst[:, b * TB + 3:b * TB + 3 + cw],
                                       pb[:, 3:3 + cw])
                    else:
                        nc.vector.tensor_copy(
                            dst[:, b * TB + 3:b * TB + 3 + cw],
                            pb[:, 3:3 + cw])

            # ---- median count-histogram over in-image R of this strip ----
            # rows: shard [114k, 114k+vrows) <-> Rrm partitions [7, 7+vrows).
            # Compute engines need partition-0-aligned accesses: stage rows
            # into cs via SBUF->SBUF DMA over +1e30 sentinels.
            cnt = cntp.tile([128, NHIST], F32, tag="cnt")
            cs = cntp.tile([128, WIMG], F32, tag="cs")
            nc.gpsimd.memset(cs[:], 1.0e30)
            nc.sync.dma_start(cs[0:vrows, :],
                              Rrm[7:7 + vrows, CPAD:CPAD + WIMG])
            junk = qpool.tile([128, WIMG], F32, tag="hif")  # reuse hif buffer
            for j in range(NHIST):
                nc.vector.tensor_scalar(
                    junk[:], cs[:],
                    float(HIST_EDGES[j]), None, OP.is_lt, OP.add,
                    accum_out=cnt[:, j:j + 1])
            nc.tensor.matmul(hist_ps[:], W_ONES, cnt[:],
                             start=(k == 0), stop=(k == NSTRIP - 1))

            # ---- store R (with halos) and Pv ----
            nc.sync.dma_start(r_d.ap()[k * STRIP:k * STRIP + rstore, :],
                              Rrm[4:4 + rstore, CPAD:CPAD + WIMG])
            nc.sync.dma_start(pv_d.ap()[k * STRIP:k * STRIP + vrows, :],
                              Pvrm[7:7 + vrows, CPAD:CPAD + WIMG])

        hsb = wpool.tile([1, NHIST], F32, tag="hsb")
        nc.scalar.copy(hsb[:], hist_ps[0:1, :])
        nc.sync.dma_start(hist_d.ap(), hsb[:])

    nc.compile()
    return nc


def _build_nc2():
    nc = bacc.Bacc("TRN2", target_bir_lowering=False, debug=False,
                   num_devices=NCORES)
    r_d = nc.dram_tensor("R_in", [RROWS, WIMG], F32, kind="ExternalInput")
    pv_d = nc.dram_tensor("Pv_in", [SHARD, WIMG], F32, kind="ExternalInput")
    m_d = nc.dram_tensor("mrep", [128, 1], F32, kind="ExternalInput")
    o_d = nc.dram_tensor("out_h", [SHARD, WIMG], F16, kind="ExternalOutput")

    PW = WIMG + 6
    with tile.TileContext(nc) as tc, ExitStack() as ctx:
        pool = ctx.enter_context(tc.tile_pool(name="p", bufs=2))
        mpool = ctx.enter_context(tc.tile_pool(name="m", bufs=1))

        mrep = mpool.tile([128, 1], F32, tag="m")
        nc.sync.dma_start(mrep[:], m_d.ap())

        for t in range(SHARD // 128):
            pvt = pool.tile([128, PW], F32, tag="pv")
            nc.gpsimd.memset(pvt[:, 0:3], 0.0)
            nc.gpsimd.memset(pvt[:, PW - 3:PW], 0.0)
            nc.sync.dma_start(pvt[:, 3:3 + WIMG],
                              pv_d.ap()[t * 128:(t + 1) * 128, :])
            rt = pool.tile([128, WIMG], F32, tag="r")
            nc.sync.dma_start(rt[:], r_d.ap()[3 + t * 128:131 + t * 128, :])

            m3 = pool.tile([128, PW], F32, tag="m3")
            nc.vector.tensor_tensor(m3[:, 1:PW - 1], pvt[:, 0:PW - 2],
                                    pvt[:, 1:PW - 1], OP.max)
            nc.vector.tensor_tensor(m3[:, 1:PW - 1], m3[:, 1:PW - 1],
                                    pvt[:, 2:PW], OP.max)
            P = pool.tile([128, WIMG], F32, tag="P")
            nc.vector.tensor_tensor(P[:], m3[:, 1:1 + WIMG],
                                    m3[:, 3:3 + WIMG], OP.max)
            nc.vector.tensor_tensor(P[:], P[:], m3[:, 5:5 + WIMG], OP.max)

            eq = pool.tile([128, WIMG], F32, tag="eq")
            nc.vector.tensor_tensor(eq[:], rt[:], P[:], OP.is_equal)
            lt = pool.tile([128, WIMG], F32, tag="lt")
            nc.vector.tensor_scalar(lt[:], P[:], mrep[:], None, OP.is_lt)
            nc.vector.tensor_tensor(eq[:], eq[:], lt[:], OP.max)
            of = pool.tile([128, WIMG], F16, tag="of")
            nc.vector.tensor_tensor(of[:], rt[:], eq[:], OP.mult)
            nc.sync.dma_start(o_d.ap()[t * 128:(t + 1) * 128, :], of[:])

    nc.compile()
    return nc


def _alloc_info(nc):
    partition_name = (nc.partition_id_tensor.name
                      if nc.partition_id_tensor else None)
    in_names, out_names, out_avals = [], [], []
    for alloc in nc.m.functions[0].allocations:
        if not isinstance(alloc, mybir.MemoryLocationSet):
            continue
        name = alloc.memorylocations[0].name
        if alloc.kind == "ExternalInput":
            if name != partition_name:
                in_names.append(name)
        elif alloc.kind == "ExternalOutput":
            out_names.append(name)
            out_avals.append(jax.core.ShapedArray(
                tuple(alloc.tensor_shape), mybir.dt.np(alloc.dtype)))
    return partition_name, in_names, out_names, out_avals


def _make_sharded(nc, mesh):
    """Cached jit of the bass_exec custom call; outputs are allocated by the
    runtime (our kernels write every element), so no donated zero buffers."""
    partition_name, in_names, out_names, out_avals = _alloc_info(nc)
    in_names_all = list(in_names)
    if partition_name:
        in_names_all.append(partition_name)

    def _body(*args):
        operands = list(args)
        if partition_name:
            operands.append(partition_id_tensor())
        return tuple(_bass_exec_p.bind(
            *operands, out_avals=tuple(out_avals),
            in_names=tuple(in_names_all), out_names=tuple(out_names),
            lowering_input_output_aliases=(), sim_require_finite=True,
            sim_require_nnan=True, nc=nc))

    fn = jax.jit(shard_map(_body, mesh=mesh,
                           in_specs=(PartitionSpec("core"),) * len(in_names),
                           out_specs=(PartitionSpec("core"),) * len(out_names),
                           check_rep=False))
    return fn, in_names, out_names, out_avals


def _get_runtime():
    if "rt" in _cache:
        return _cache["rt"]
    install_neuronx_cc_hook()
    devices = jax.devices()[:NCORES]
    mesh = Mesh(np.asarray(devices), ("core",))
    sh = NamedSharding(mesh, PartitionSpec("core"))
    nc1 = _build_nc1()
    nc2 = _build_nc2()
    f1, in1, outn1, _ = _make_sharded(nc1, mesh)
    f2, in2, outn2, _ = _make_sharded(nc2, mesh)

    # constant per-core row masks (in-image indicator per xpad row)
    NR = NSTRIP * STRIP + 14   # 1040
    mk_g = np.zeros((NCORES * NR, 1), np.float32)
    for c in range(NCORES):
        if c % 2 == 0:
            mk_g[c * NR + 7:(c + 1) * NR] = 1.0
        else:
            mk_g[c * NR:c * NR + 1031] = 1.0
    _cache["rt"] = dict(mesh=mesh, sh=sh, f1=f1, in1=in1, f2=f2, in2=in2,
                        mk_g=mk_g, outn1=outn1)
    return _cache["rt"]


def _quantize_core(x_c):
    """x_c (1024,2048) f32 -> (hi int16, lo uint8, first7 f32, last7 f32)."""
    buf = x_c * np.float32(QINV)
    np.rint(buf, out=buf)
    q = buf.astype(np.int32)
    np.clip(q, -8388608, 8388607, out=q)
    hi = (q >> 8).astype(np.int16)
    lo = q.astype(np.uint8)
    return hi, lo, q[0:7].astype(np.float32), q[1017:1024].astype(np.float32)


def _host_maxpool7_pad(a, pad_val):
    Hh, Ww = a.shape
    pad = np.full((Hh + 6, Ww + 6), pad_val, dtype=np.float32)
    pad[3:-3, 3:-3] = a
    A = np.full((Hh + 6, Ww), pad_val, dtype=np.float32)
    for d in range(7):
        np.maximum(A, pad[:, d:d + Ww], out=A)
    P = np.full((Hh, Ww), pad_val, dtype=np.float32)
    for d in range(7):
        np.maximum(P, A[d:d + Hh], out=P)
    return P


def _host_fallback(r_dev):
    """Exact host pipeline from the device R (used when the tuned median
    histogram range misses or the median is non-positive)."""
    Rb = np.asarray(r_dev).reshape(NCORES, RROWS, WIMG)[:, 3:3 + SHARD, :]
    R = Rb.reshape(4, 2, SHARD, WIMG).reshape(4, H, WIMG)
    M = np.partition(R.ravel(), K0)[K0]
    out = np.empty((4, 1, H, WIMG), np.float32)
    for i in range(4):
        thr = np.where(R[i] < M, np.float32(0.0), R[i])
        pooled = _host_maxpool7_pad(thr, -np.inf)
        out[i, 0] = np.where(thr == pooled, np.float32(1.0),
                             np.float32(0.0)) * R[i]
    return out


def _run_full(x):
    """Full pipeline: host numpy x -> final full-shape fp32 output."""
    rt = _get_runtime()
    x = np.ascontiguousarray(np.asarray(x, dtype=np.float32))
    x8 = x.reshape(NCORES * SHARD, WIMG)
    devices = list(rt["mesh"].devices.flat)
    sh = rt["sh"]

    # per-core quantize -> async per-device upload pipeline: quantizing
    # chunk c+1 overlaps the (link-bound) transfer of chunk c
    hi_dev, lo_dev, first7, last7 = [], [], [], []
    for c in range(NCORES):
        hi, lo, f7, l7 = _quantize_core(x8[c * SHARD:(c + 1) * SHARD])
        hi_dev.append(jax.device_put(hi, devices[c]))
        lo_dev.append(jax.device_put(lo, devices[c]))
        first7.append(f7)
        last7.append(l7)
    hi_g = jax.make_array_from_single_device_arrays(
        (NCORES * SHARD, WIMG), sh, hi_dev)
    lo_g = jax.make_array_from_single_device_arrays(
        (NCORES * SHARD, WIMG), sh, lo_dev)

    halo_g = np.zeros((NCORES * 14, WIMG), np.float32)
    for c in range(NCORES):
        if c % 2 == 1:
            halo_g[c * 14:c * 14 + 7] = last7[c - 1]   # image rows 1017..1024
        else:
            halo_g[c * 14 + 7:c * 14 + 14] = first7[c + 1]  # rows 1024..1031

    ins1 = {"xhi": hi_g, "xlo": lo_g, "halo": halo_g, "rowmask": rt["mk_g"]}
    args1 = [ins1[nm] for nm in rt["in1"]]
    outs1 = rt["f1"](*args1)
    byname1 = dict(zip(rt["outn1"], outs1))
    r_dev, pv_dev = byname1["R_buf"], byname1["Pv_buf"]

    def run_f2(Mp):
        mrep = np.full((NCORES * 128, 1), Mp, np.float32)
        ins2 = {"R_in": r_dev, "Pv_in": pv_dev, "mrep": mrep}
        return rt["f2"](*[ins2[nm] for nm in rt["in2"]])[0]

    # value speculation: dispatch f2 with the memoized median of the previous
    # call (async) while the histogram is fetched and verified concurrently.
    m_spec = _cache.get("m_spec")
    out_spec = run_f2(m_spec) if m_spec is not None else None

    hist = np.asarray(byname1["hist"])          # (8, 512) float32
    counts = hist.reshape(NCORES, NHIST).sum(axis=0).astype(np.int64)
    if not (counts[0] <= K0 and counts[-1] > K0):
        return _host_fallback(r_dev)
    j = int(np.searchsorted(counts > K0, True)) - 1
    Mp = float(HIST_EDGES[j])
    if not (Mp > 0.0):
        return _host_fallback(r_dev)
    _cache["m_spec"] = Mp

    out_dev = out_spec if (out_spec is not None and m_spec == Mp) \
        else run_f2(Mp)

    # fetch the 8 fp16 shards in parallel, converting to f32 as they land
    out = np.empty((4, 1, H, WIMG), np.float32)
    oflat = out.reshape(NCORES, SHARD, WIMG)
    try:
        shards = sorted(out_dev.addressable_shards,
                        key=lambda s: s.index[0].start or 0)
        assert len(shards) == NCORES

        def fetch(c):
            oflat[c] = np.asarray(shards[c].data)
        with ThreadPoolExecutor(8) as ex:
            list(ex.map(fetch, range(NCORES)))
    except Exception:
        oflat[:] = np.asarray(out_dev).reshape(NCORES, SHARD, WIMG)
    return out


def run_device(x, **_):
    out = _run_full(x)
    return out, None


def kernel(x, sobel_kernel=None, gauss_kernel=None, **_):
    return _run_full(x)
